# revision 18
# baseline (speedup 1.0000x reference)
"""GCNII (8-layer GCN2Conv stack) on 8 Trainium2 NeuronCores.

Strategy (1D graph parallel over destination nodes):
  - Core c owns destination rows [c*NS, (c+1)*NS) of every layer's output.
  - Edges are partitioned by dst, sorted by dst, grouped into per-128-dst-node
    "blocks" padded to 128-edge chunks (schedule shared across cores, SPMD);
    blocks are grouped into granules of GRAN blocks. Within each (block,
    src-half) group edges are ordered by source row so consecutive gather
    descriptors hit nearby HBM addresses.
  - One-hot selector tiles S[e, j] = (j == dstloc[e]) * 0.9*w[e] are constant
    across layers: precomputed on the HOST (bf16) and streamed from DRAM once
    per granule per layer (sync-engine DMA) instead of being rebuilt on the
    vector engine per chunk per layer.
  - Gathers are granule-batched: per (granule, src-half) one gpsimd.dma_gather
    pulls all the granule's source rows (128 rows x 256B, bf16) from the
    full-h DRAM buffer into SBUF; the lo/hi src-half split keeps gather
    indices within int16.
  - Per chunk: tensor-engine matmul aggT += V.T @ S accumulated in PSUM
    (feature-major [feat, dst]); per block: mT = aggT + 0.1*x0T (vector),
    node-major h' = relu(mT.T @ W_hat_l) via one matmul (W_hat_l =
    beta_l*W_l + (1-beta_l)*I folded on host; the GCNII (1-alpha) factor is
    folded into edge weights) + scalar-engine relu, DMA to the layer's
    shard buffer.
  - AllGather of the 6250-row shard between layers gives every core the full
    h for the next layer's gathers. Final layer writes the external output
    directly (host concatenates shards).
"""

import numpy as np

N = 50000
E = 800000
D = 128
L = 8
ALPHA = 0.1
THETA = 0.5
NCORES = 8
P = 128
LO_N = 32768
GRAN = 4

_NEFF_CACHE = {}
_LAST_IN_MAPS = None
_GATHER_MODE = "batch"
_SEL_DMA = True
_MM_ON = True
_AG_OFF = False
_H_BF16 = True
_VP_BUFS = 3
_SP_BUFS = 3
_PS_BUFS = 4
_SORT_SRC = True


def _preprocess(edge_index, edge_weight, n, ns, nb, ncores):
    import ml_dtypes

    src = np.asarray(edge_index[0], dtype=np.int64)
    dst = np.asarray(edge_index[1], dtype=np.int64)
    w = np.asarray(edge_weight, dtype=np.float32) * (1.0 - ALPHA)

    order = np.argsort(dst, kind="stable")
    src_s, dst_s, w_s = src[order], dst[order], w[order]

    core = dst_s // ns
    local = dst_s - core * ns
    block = local // P
    dstloc = (local - block * P).astype(np.int64)
    hi = (src_s >= LO_N).astype(np.int64)

    ngran = -(-nb // GRAN)

    cbh = (core * nb + block) * 2 + hi
    counts = np.bincount(cbh, minlength=ncores * nb * 2).reshape(ncores, nb, 2)
    cpg = -(-counts.max(axis=0) // P)
    cpg = np.maximum(cpg, 1)
    nch = int(cpg.sum())

    group_koff = np.zeros((nb, 2), dtype=np.int64)
    gathers = []
    k = 0
    for g in range(ngran):
        b0, b1 = g * GRAN, min((g + 1) * GRAN, nb)
        for h in range(2):
            k0 = k
            for b in range(b0, b1):
                group_koff[b, h] = k
                k += cpg[b, h]
            gathers.append((g, h, int(k0), int(k - k0)))
    assert k == nch
    block_chunks = {
        b: [(int(group_koff[b, 0]), int(cpg[b, 0])),
            (int(group_koff[b, 1]), int(cpg[b, 1]))]
        for b in range(nb)
    }

    sel_arrs, idx_arrs = [], []
    core_starts = np.searchsorted(core, np.arange(ncores + 1))
    for c in range(ncores):
        s0, s1 = core_starts[c], core_starts[c + 1]
        blk = block[s0:s1]
        bstart = np.searchsorted(blk, np.arange(nb + 1)) + s0
        da = np.zeros(nch * P, dtype=np.int64)
        wa = np.zeros(nch * P, dtype=np.float32)
        i16 = np.zeros(nch * P, dtype=np.int16)
        for b in range(nb):
            e0, e1 = bstart[b], bstart[b + 1]
            seg_src = src_s[e0:e1]
            seg_hi = seg_src >= LO_N
            for h in range(2):
                m = seg_hi if h else ~seg_hi
                gsrc = seg_src[m]
                if _SORT_SRC:
                    o = np.argsort(gsrc, kind="stable")
                else:
                    o = np.arange(len(gsrc))
                cnt = len(gsrc)
                pos = group_koff[b, h] * P
                da[pos:pos + cnt] = dstloc[e0:e1][m][o]
                wa[pos:pos + cnt] = w_s[e0:e1][m][o]
                i16[pos:pos + cnt] = (gsrc[o] - (LO_N if h else 0)).astype(
                    np.int16)
        s3 = np.zeros((nch, P, P), dtype=np.float32)
        kk = np.repeat(np.arange(nch), P)
        pp = np.tile(np.arange(P), nch)
        s3[kk, pp, da] = wa
        sel = np.ascontiguousarray(
            s3.transpose(1, 0, 2).reshape(P, nch * P)).astype(ml_dtypes.bfloat16)
        sel_arrs.append(sel)
        ia = np.zeros((P, nch * 8), dtype=np.int16)
        for (_, _, k0, m) in gathers:
            sub = i16[k0 * P:(k0 + m) * P]
            ia[:16, k0 * 8:(k0 + m) * 8] = sub.reshape(m * 8, 16).T
        ia = np.tile(ia[:16], (8, 1))
        idx_arrs.append(np.ascontiguousarray(ia))

    return sel_arrs, idx_arrs, nch, gathers, block_chunks


def _build(nc, *, n, ns, nb, nch, gathers, block_chunks, n_layers):
    import concourse.mybir as mybir
    import concourse.tile as tile
    from concourse.masks import make_identity

    f32 = mybir.dt.float32
    fh = mybir.dt.bfloat16 if _H_BF16 else f32

    x_t = nc.dram_tensor("x_shard", [ns, D], f32, kind="ExternalInput")
    wlin_t = nc.dram_tensor("w_lin", [D, D], f32, kind="ExternalInput")
    blin_t = nc.dram_tensor("b_lin", [D], f32, kind="ExternalInput")
    what_t = nc.dram_tensor("w_hat", [n_layers, D, D], f32, kind="ExternalInput")
    sel_t = nc.dram_tensor("sel", [P, nch * P], fh, kind="ExternalInput")
    idx16_t = nc.dram_tensor("idx16", [P, nch * 8], mybir.dt.int16,
                             kind="ExternalInput")
    out_t = nc.dram_tensor("h_out", [ns, D], f32, kind="ExternalOutput")

    hbuf = [nc.dram_tensor(f"h_full{i}", [n, D], fh, addr_space="Shared")
            for i in range(2)]
    shbuf = [nc.dram_tensor(f"h_shard{i}", [ns, D], fh) for i in range(2)]
    rg = [list(range(NCORES))]

    n_full = ns // P
    last = ns - n_full * P
    blk_rows = [P] * n_full + ([last] if last else [])
    assert len(blk_rows) == nb

    gran_of = {}
    for (g, h, k0, m) in gathers:
        gran_of.setdefault(g, []).append((h, k0, m))
    ngran = len(gran_of)
    m_max = [max(m for (h2, _, m) in sum(gran_of.values(), []) if h2 == h)
             for h in range(2)]
    gsel = []
    for g in range(ngran):
        ks = [(k0, m) for (_, k0, m) in gran_of[g]]
        gsel.append((min(k for k, _ in ks), sum(m for _, m in ks)))
    msel_max = max(m for _, m in gsel)

    with tile.TileContext(nc) as tc:
        with (
            tc.tile_pool(name="res", bufs=1) as res,
            tc.tile_pool(name="xp", bufs=3) as xp,
            tc.tile_pool(name="vp", bufs=_VP_BUFS) as vp,
            tc.tile_pool(name="sp", bufs=_SP_BUFS) as sp,
            tc.tile_pool(name="mp", bufs=4) as mp,
            tc.tile_pool(name="hp", bufs=6) as hp,
            tc.tile_pool(name="ps", bufs=2, space="PSUM") as ps,
            tc.tile_pool(name="psa", bufs=_PS_BUFS, space="PSUM") as psa,
        ):
            ident = res.tile([P, P], f32, tag="ident")
            make_identity(nc, ident[:])

            wlin_s = res.tile([P, D], f32, tag="wlin")
            nc.sync.dma_start(out=wlin_s[:], in_=wlin_t[:])
            blin_s = res.tile([P, 1], f32, tag="blin")
            nc.sync.dma_start(out=blin_s[:], in_=blin_t[:, None])
            what_s = res.tile([P, n_layers * D], f32, tag="what")
            for l in range(n_layers):
                nc.sync.dma_start(out=what_s[:, l * D:(l + 1) * D],
                                  in_=what_t[l, :, :])
            idx16 = res.tile([P, nch * 8], mybir.dt.int16, tag="idx16")
            nc.sync.dma_start(out=idx16[:], in_=idx16_t[:])

            x0sT = res.tile([P, ns], f32, tag="x0sT")

            sel_const = None
            if not _SEL_DMA:
                sel_const = res.tile([P, P], fh, tag="selc")
                nc.vector.memset(sel_const[:], 0.0)
            v_const = None
            if _GATHER_MODE == "off":
                v_const = res.tile([P, 1, D], fh, tag="vc")
                nc.vector.memset(v_const[:], 0.0)

            for b in range(nb):
                rows = blk_rows[b]
                r0 = b * P
                xb = xp.tile([P, D], f32, tag="xb")
                nc.sync.dma_start(out=xb[:rows, :], in_=x_t[r0:r0 + rows, :])
                xbT_ps = ps.tile([P, P], f32, tag="tr", space="PSUM")
                nc.tensor.transpose(out=xbT_ps[:, :rows], in_=xb[:rows, :],
                                    identity=ident[:rows, :rows])
                xbT = xp.tile([P, P], f32, tag="xbT")
                nc.vector.tensor_copy(out=xbT[:, :rows], in_=xbT_ps[:, :rows])
                ps2 = ps.tile([P, P], f32, tag="dense", space="PSUM")
                nc.tensor.matmul(out=ps2[:, :rows], lhsT=wlin_s[:],
                                 rhs=xbT[:, :rows], start=True, stop=True)
                x0Tb = xp.tile([P, P], f32, tag="x0Tb")
                nc.scalar.activation(out=x0Tb[:, :rows], in_=ps2[:, :rows],
                                     func=mybir.ActivationFunctionType.Relu,
                                     bias=blin_s[:, :1], scale=1.0)
                nc.vector.tensor_scalar(
                    out=x0sT[:, r0:r0 + rows], in0=x0Tb[:, :rows],
                    scalar1=ALPHA, scalar2=None, op0=mybir.AluOpType.mult)
                x0_ps = ps.tile([P, P], f32, tag="tr", space="PSUM")
                nc.tensor.transpose(out=x0_ps[:rows, :], in_=x0Tb[:, :rows],
                                    identity=ident[:])
                x0b = hp.tile([P, D], fh, tag="hb")
                nc.vector.tensor_copy(out=x0b[:rows, :], in_=x0_ps[:rows, :])
                nc.sync.dma_start(out=shbuf[0][r0:r0 + rows, :],
                                  in_=x0b[:rows, :])

            if _AG_OFF:
                nc.sync.dma_start(out=hbuf[0][0:ns, :], in_=shbuf[0][:])
            else:
                nc.gpsimd.collective_compute(
                    "AllGather", mybir.AluOpType.bypass, replica_groups=rg,
                    ins=[shbuf[0][:]], outs=[hbuf[0][:]])

            gq = 0
            for l in range(n_layers):
                h_cur = hbuf[l % 2]
                is_last = l == n_layers - 1
                for g in range(ngran):
                    b0, b1 = g * GRAN, min((g + 1) * GRAN, nb)
                    k0s, msel = gsel[g]
                    if _SEL_DMA:
                        selg = sp.tile([P, msel_max * P], fh, tag="sel")
                        nc.sync.dma_start(
                            out=selg[:, :msel * P],
                            in_=sel_t[:, k0s * P:(k0s + msel) * P])
                    vts = {}
                    for (h, k0, m) in gran_of[g]:
                        vt = vp.tile([P, m_max[h], D], fh, tag=f"v{h}")
                        vts[h] = (vt, k0)
                        if _GATHER_MODE == "batch":
                            in_ap = (h_cur[LO_N:n, :] if h
                                     else h_cur[0:LO_N, :])
                            nc.gpsimd.dma_gather(
                                vt[:, :m, :], in_ap,
                                idx16[:, k0 * 8:(k0 + m) * 8],
                                num_idxs=m * P, num_idxs_reg=m * P,
                                elem_size=D, single_packet=False,
                                queue_num=gq % 4)
                            gq += 1
                    for b in range(b0, b1):
                        rows = blk_rows[b]
                        r0 = b * P
                        nk = sum(m for _, m in block_chunks[b])
                        aggT = psa.tile([P, P], f32, tag="agg", space="PSUM")
                        ki = 0
                        for h in range(2):
                            kb, mb = block_chunks[b][h]
                            vt, kg = vts[h]
                            for j in range(mb):
                                kchunk = kb + j
                                if _SEL_DMA:
                                    s_ap = selg[:, (kchunk - k0s) * P:
                                                (kchunk - k0s + 1) * P]
                                else:
                                    s_ap = sel_const[:]
                                v_ap = (v_const[:, 0, :] if v_const is not None
                                        else vt[:, kchunk - kg, :])
                                if _MM_ON:
                                    nc.tensor.matmul(
                                        out=aggT[:], lhsT=v_ap, rhs=s_ap,
                                        start=(ki == 0), stop=(ki == nk - 1))
                                ki += 1
                        if not _MM_ON:
                            nc.vector.memset(aggT[:], 0.0)
                        mT = mp.tile([P, P], f32, tag="mT")
                        nc.vector.tensor_tensor(
                            out=mT[:, :rows], in0=aggT[:, :rows],
                            in1=x0sT[:, r0:r0 + rows], op=mybir.AluOpType.add)
                        ps2 = ps.tile([P, P], f32, tag="dense", space="PSUM")
                        nc.tensor.matmul(out=ps2[:rows, :], lhsT=mT[:, :rows],
                                         rhs=what_s[:, l * D:(l + 1) * D],
                                         start=True, stop=True)
                        hb = hp.tile([P, D], f32 if is_last else fh, tag="hbo")
                        nc.scalar.activation(
                            out=hb[:rows, :], in_=ps2[:rows, :],
                            func=mybir.ActivationFunctionType.Relu)
                        dst_dram = out_t if is_last else shbuf[(l + 1) % 2]
                        nc.sync.dma_start(out=dst_dram[r0:r0 + rows, :],
                                          in_=hb[:rows, :])
                if not is_last:
                    if _AG_OFF:
                        nc.sync.dma_start(out=hbuf[(l + 1) % 2][0:ns, :],
                                          in_=shbuf[(l + 1) % 2][:])
                    else:
                        nc.gpsimd.collective_compute(
                            "AllGather", mybir.AluOpType.bypass,
                            replica_groups=rg,
                            ins=[shbuf[(l + 1) % 2][:]],
                            outs=[hbuf[(l + 1) % 2][:]])
    return nc


def _run(inputs, *, n, e, n_layers, ncores=NCORES):
    import concourse.bacc as bacc
    from concourse.bass_utils import run_bass_kernel_spmd

    x = np.asarray(inputs["x"], dtype=np.float32)
    edge_weight = np.asarray(inputs["edge_weight"], dtype=np.float32)
    w_lin = np.asarray(inputs["W_lin"], dtype=np.float32)
    b_lin = np.asarray(inputs["b_lin"], dtype=np.float32)
    w_convs = np.asarray(inputs["W_convs"], dtype=np.float32)
    edge_index = np.asarray(inputs["edge_index"])

    ns = n // ncores
    nb = -(-ns // P)

    betas = np.log(THETA / np.arange(1, n_layers + 1) + 1.0).astype(np.float32)
    eye = np.eye(D, dtype=np.float32)
    w_hat = np.stack([betas[l] * w_convs[l] + (1.0 - betas[l]) * eye
                      for l in range(n_layers)]).astype(np.float32)

    sel_arrs, idx_arrs, nch, gathers, block_chunks = \
        _preprocess(edge_index, edge_weight, n, ns, nb, ncores)

    key = (n, e, n_layers, nch, _SORT_SRC)
    if key not in _NEFF_CACHE:
        nc = bacc.Bacc("TRN2", target_bir_lowering=False, debug=False,
                       num_devices=ncores, num_swdge_queues=4,
                       dynamic_dma_scratch_size=32768)
        _build(nc, n=n, ns=ns, nb=nb, nch=nch, gathers=gathers,
               block_chunks=block_chunks, n_layers=n_layers)
        nc.compile()
        _NEFF_CACHE[key] = nc
    nc = _NEFF_CACHE[key]

    in_maps = []
    for c in range(ncores):
        in_maps.append({
            "x_shard": np.ascontiguousarray(x[c * ns:(c + 1) * ns]),
            "w_lin": w_lin, "b_lin": b_lin, "w_hat": w_hat,
            "sel": sel_arrs[c],
            "idx16": idx_arrs[c],
        })
    global _LAST_IN_MAPS
    _LAST_IN_MAPS = in_maps
    res = run_bass_kernel_spmd(nc, in_maps, list(range(ncores)))
    out = np.concatenate([res.results[c]["h_out"] for c in range(ncores)],
                         axis=0)
    return out


def kernel(x, edge_weight, W_lin, b_lin, W_convs, edge_index):
    return _run(
        dict(x=x, edge_weight=edge_weight, W_lin=W_lin, b_lin=b_lin,
             W_convs=W_convs, edge_index=edge_index),
        n=N, e=E, n_layers=L)


# revision 20
# speedup vs baseline: 1.0093x; 1.0093x over previous
"""GCNII (8-layer GCN2Conv stack) on 8 Trainium2 NeuronCores.

Strategy (1D graph parallel over destination nodes):
  - Core c owns destination rows [c*NS, (c+1)*NS) of every layer's output.
  - Edges are partitioned by dst, sorted by dst, grouped into per-128-dst-node
    "blocks" padded to 128-edge chunks (schedule shared across cores, SPMD);
    blocks are grouped into granules of GRAN blocks. Within each (block,
    src-half) group edges are ordered by source row so consecutive gather
    descriptors hit nearby HBM addresses.
  - One-hot selector tiles S[e, j] = (j == dstloc[e]) * 0.9*w[e] are constant
    across layers: precomputed on the HOST (bf16) and streamed from DRAM once
    per granule per layer (sync-engine DMA) instead of being rebuilt on the
    vector engine per chunk per layer.
  - Gathers are granule-batched: per (granule, src-half) one gpsimd.dma_gather
    pulls all the granule's source rows (128 rows x 256B, bf16) from the
    full-h DRAM buffer into SBUF; the lo/hi src-half split keeps gather
    indices within int16.
  - Per chunk: tensor-engine matmul aggT += V.T @ S accumulated in PSUM
    (feature-major [feat, dst]); per block: mT = aggT + 0.1*x0T (vector),
    node-major h' = relu(mT.T @ W_hat_l) via one matmul (W_hat_l =
    beta_l*W_l + (1-beta_l)*I folded on host; the GCNII (1-alpha) factor is
    folded into edge weights) + scalar-engine relu, DMA to the layer's
    shard buffer.
  - AllGather of the 6250-row shard between layers gives every core the full
    h for the next layer's gathers. Final layer writes the external output
    directly (host concatenates shards).
"""

import numpy as np

N = 50000
E = 800000
D = 128
L = 8
ALPHA = 0.1
THETA = 0.5
NCORES = 8
P = 128
LO_N = 32768
GRAN = 4

_NEFF_CACHE = {}
_LAST_IN_MAPS = None
_GATHER_MODE = "batch"
_SEL_DMA = True
_MM_ON = True
_AG_OFF = False
_H_BF16 = True
_VP_BUFS = 3
_SP_BUFS = 3
_PS_BUFS = 4
_SORT_SRC = True


def _preprocess(edge_index, edge_weight, n, ns, nb, ncores):
    import ml_dtypes

    src = np.asarray(edge_index[0], dtype=np.int64)
    dst = np.asarray(edge_index[1], dtype=np.int64)
    w = np.asarray(edge_weight, dtype=np.float32) * (1.0 - ALPHA)

    order = np.argsort(dst, kind="stable")
    src_s, dst_s, w_s = src[order], dst[order], w[order]

    core = dst_s // ns
    local = dst_s - core * ns
    block = local // P
    dstloc = (local - block * P).astype(np.int64)
    hi = (src_s >= LO_N).astype(np.int64)

    ngran = -(-nb // GRAN)

    cbh = (core * nb + block) * 2 + hi
    counts = np.bincount(cbh, minlength=ncores * nb * 2).reshape(ncores, nb, 2)
    cpg = -(-counts.max(axis=0) // P)
    cpg = np.maximum(cpg, 1)
    nch = int(cpg.sum())

    group_koff = np.zeros((nb, 2), dtype=np.int64)
    gathers = []
    k = 0
    for g in range(ngran):
        b0, b1 = g * GRAN, min((g + 1) * GRAN, nb)
        for h in range(2):
            k0 = k
            for b in range(b0, b1):
                group_koff[b, h] = k
                k += cpg[b, h]
            gathers.append((g, h, int(k0), int(k - k0)))
    assert k == nch
    block_chunks = {
        b: [(int(group_koff[b, 0]), int(cpg[b, 0])),
            (int(group_koff[b, 1]), int(cpg[b, 1]))]
        for b in range(nb)
    }

    sel_arrs, idx_arrs = [], []
    core_starts = np.searchsorted(core, np.arange(ncores + 1))
    for c in range(ncores):
        s0, s1 = core_starts[c], core_starts[c + 1]
        blk = block[s0:s1]
        bstart = np.searchsorted(blk, np.arange(nb + 1)) + s0
        da = np.zeros(nch * P, dtype=np.int64)
        wa = np.zeros(nch * P, dtype=np.float32)
        i16 = np.zeros(nch * P, dtype=np.int16)
        for b in range(nb):
            e0, e1 = bstart[b], bstart[b + 1]
            seg_src = src_s[e0:e1]
            seg_hi = seg_src >= LO_N
            for h in range(2):
                m = seg_hi if h else ~seg_hi
                gsrc = seg_src[m]
                if _SORT_SRC:
                    o = np.argsort(gsrc, kind="stable")
                else:
                    o = np.arange(len(gsrc))
                cnt = len(gsrc)
                pos = group_koff[b, h] * P
                da[pos:pos + cnt] = dstloc[e0:e1][m][o]
                wa[pos:pos + cnt] = w_s[e0:e1][m][o]
                i16[pos:pos + cnt] = (gsrc[o] - (LO_N if h else 0)).astype(
                    np.int16)
        s3 = np.zeros((nch, P, P), dtype=np.float32)
        kk = np.repeat(np.arange(nch), P)
        pp = np.tile(np.arange(P), nch)
        s3[kk, pp, da] = wa
        sel = np.ascontiguousarray(
            s3.transpose(1, 0, 2).reshape(P, nch * P)).astype(ml_dtypes.bfloat16)
        sel_arrs.append(sel)
        ia = np.zeros((P, nch * 8), dtype=np.int16)
        for (_, _, k0, m) in gathers:
            sub = i16[k0 * P:(k0 + m) * P]
            ia[:16, k0 * 8:(k0 + m) * 8] = sub.reshape(m * 8, 16).T
        ia = np.tile(ia[:16], (8, 1))
        idx_arrs.append(np.ascontiguousarray(ia))

    return sel_arrs, idx_arrs, nch, gathers, block_chunks


def _build(nc, *, n, ns, nb, nch, gathers, block_chunks, n_layers):
    import concourse.mybir as mybir
    import concourse.tile as tile
    from concourse.masks import make_identity

    f32 = mybir.dt.float32
    fh = mybir.dt.bfloat16 if _H_BF16 else f32

    x_t = nc.dram_tensor("x_shard", [ns, D], f32, kind="ExternalInput")
    wlin_t = nc.dram_tensor("w_lin", [D, D], f32, kind="ExternalInput")
    blin_t = nc.dram_tensor("b_lin", [D], f32, kind="ExternalInput")
    what_t = nc.dram_tensor("w_hat", [n_layers, D, D], f32, kind="ExternalInput")
    sel_t = nc.dram_tensor("sel", [P, nch * P], fh, kind="ExternalInput")
    idx16_t = nc.dram_tensor("idx16", [P, nch * 8], mybir.dt.int16,
                             kind="ExternalInput")
    out_t = nc.dram_tensor("h_out", [ns, D], f32, kind="ExternalOutput")

    hbuf = [nc.dram_tensor(f"h_full{i}", [n, D], fh, addr_space="Shared")
            for i in range(2)]
    shbuf = [nc.dram_tensor(f"h_shard{i}", [ns, D], fh) for i in range(2)]
    rg = [list(range(NCORES))]

    n_full = ns // P
    last = ns - n_full * P
    blk_rows = [P] * n_full + ([last] if last else [])
    assert len(blk_rows) == nb

    gran_of = {}
    for (g, h, k0, m) in gathers:
        gran_of.setdefault(g, []).append((h, k0, m))
    ngran = len(gran_of)
    m_max = [max(m for (h2, _, m) in sum(gran_of.values(), []) if h2 == h)
             for h in range(2)]
    gsel = []
    for g in range(ngran):
        ks = [(k0, m) for (_, k0, m) in gran_of[g]]
        gsel.append((min(k for k, _ in ks), sum(m for _, m in ks)))
    msel_max = max(m for _, m in gsel)

    with tile.TileContext(nc) as tc:
        with (
            tc.tile_pool(name="res", bufs=1) as res,
            tc.tile_pool(name="xp", bufs=3) as xp,
            tc.tile_pool(name="vp", bufs=_VP_BUFS) as vp,
            tc.tile_pool(name="sp", bufs=_SP_BUFS) as sp,
            tc.tile_pool(name="mp", bufs=4) as mp,
            tc.tile_pool(name="hp", bufs=6) as hp,
            tc.tile_pool(name="ps", bufs=2, space="PSUM") as ps,
            tc.tile_pool(name="psa", bufs=_PS_BUFS, space="PSUM") as psa,
        ):
            ident = res.tile([P, P], f32, tag="ident")
            make_identity(nc, ident[:])

            wlin_s = res.tile([P, D], f32, tag="wlin")
            nc.sync.dma_start(out=wlin_s[:], in_=wlin_t[:])
            blin_s = res.tile([P, 1], f32, tag="blin")
            nc.sync.dma_start(out=blin_s[:], in_=blin_t[:, None])
            what_s = res.tile([P, n_layers * D], f32, tag="what")
            for l in range(n_layers):
                nc.sync.dma_start(out=what_s[:, l * D:(l + 1) * D],
                                  in_=what_t[l, :, :])
            idx16 = res.tile([P, nch * 8], mybir.dt.int16, tag="idx16")
            nc.sync.dma_start(out=idx16[:], in_=idx16_t[:])

            x0sT = res.tile([P, ns], f32, tag="x0sT")

            sel_const = None
            if not _SEL_DMA:
                sel_const = res.tile([P, P], fh, tag="selc")
                nc.vector.memset(sel_const[:], 0.0)
            v_const = None
            if _GATHER_MODE == "off":
                v_const = res.tile([P, 1, D], fh, tag="vc")
                nc.vector.memset(v_const[:], 0.0)

            for b in range(nb):
                rows = blk_rows[b]
                r0 = b * P
                xb = xp.tile([P, D], f32, tag="xb")
                nc.sync.dma_start(out=xb[:rows, :], in_=x_t[r0:r0 + rows, :])
                xbT_ps = ps.tile([P, P], f32, tag="tr", space="PSUM")
                nc.tensor.transpose(out=xbT_ps[:, :rows], in_=xb[:rows, :],
                                    identity=ident[:rows, :rows])
                xbT = xp.tile([P, P], f32, tag="xbT")
                nc.vector.tensor_copy(out=xbT[:, :rows], in_=xbT_ps[:, :rows])
                ps2 = ps.tile([P, P], f32, tag="dense", space="PSUM")
                nc.tensor.matmul(out=ps2[:, :rows], lhsT=wlin_s[:],
                                 rhs=xbT[:, :rows], start=True, stop=True)
                x0Tb = xp.tile([P, P], f32, tag="x0Tb")
                nc.scalar.activation(out=x0Tb[:, :rows], in_=ps2[:, :rows],
                                     func=mybir.ActivationFunctionType.Relu,
                                     bias=blin_s[:, :1], scale=1.0)
                nc.vector.tensor_scalar(
                    out=x0sT[:, r0:r0 + rows], in0=x0Tb[:, :rows],
                    scalar1=ALPHA, scalar2=None, op0=mybir.AluOpType.mult)
                x0_ps = ps.tile([P, P], f32, tag="tr", space="PSUM")
                nc.tensor.transpose(out=x0_ps[:rows, :], in_=x0Tb[:, :rows],
                                    identity=ident[:])
                x0b = hp.tile([P, D], fh, tag="hb")
                nc.vector.tensor_copy(out=x0b[:rows, :], in_=x0_ps[:rows, :])
                nc.sync.dma_start(out=shbuf[0][r0:r0 + rows, :],
                                  in_=x0b[:rows, :])

            if _AG_OFF:
                nc.sync.dma_start(out=hbuf[0][0:ns, :], in_=shbuf[0][:])
            else:
                nc.gpsimd.collective_compute(
                    "AllGather", mybir.AluOpType.bypass, replica_groups=rg,
                    ins=[shbuf[0][:]], outs=[hbuf[0][:]])

            gq = 0
            for l in range(n_layers):
                h_cur = hbuf[l % 2]
                is_last = l == n_layers - 1
                for g in range(ngran):
                    b0, b1 = g * GRAN, min((g + 1) * GRAN, nb)
                    k0s, msel = gsel[g]
                    if _SEL_DMA:
                        selg = sp.tile([P, msel_max * P], fh, tag="sel")
                        nc.sync.dma_start(
                            out=selg[:, :msel * P],
                            in_=sel_t[:, k0s * P:(k0s + msel) * P])
                    vts = {}
                    for (h, k0, m) in gran_of[g]:
                        vt = vp.tile([P, m_max[h], D], fh, tag=f"v{h}")
                        vts[h] = (vt, k0)
                        if _GATHER_MODE == "batch":
                            in_ap = (h_cur[LO_N:n, :] if h
                                     else h_cur[0:LO_N, :])
                            nc.gpsimd.dma_gather(
                                vt[:, :m, :], in_ap,
                                idx16[:, k0 * 8:(k0 + m) * 8],
                                num_idxs=m * P, num_idxs_reg=m * P,
                                elem_size=D, single_packet=False,
                                queue_num=gq % 4)
                            gq += 1
                    for b in range(b0, b1):
                        rows = blk_rows[b]
                        r0 = b * P
                        nk = sum(m for _, m in block_chunks[b])
                        aggT = psa.tile([P, P], f32, tag="agg", space="PSUM")
                        ki = 0
                        for h in range(2):
                            kb, mb = block_chunks[b][h]
                            vt, kg = vts[h]
                            for j in range(mb):
                                kchunk = kb + j
                                if _SEL_DMA:
                                    s_ap = selg[:, (kchunk - k0s) * P:
                                                (kchunk - k0s + 1) * P]
                                else:
                                    s_ap = sel_const[:]
                                v_ap = (v_const[:, 0, :] if v_const is not None
                                        else vt[:, kchunk - kg, :])
                                if _MM_ON:
                                    nc.tensor.matmul(
                                        out=aggT[:], lhsT=v_ap, rhs=s_ap,
                                        start=(ki == 0), stop=(ki == nk - 1))
                                ki += 1
                        if not _MM_ON:
                            nc.vector.memset(aggT[:], 0.0)
                        mT = mp.tile([P, P], f32, tag="mT")
                        nc.vector.tensor_tensor(
                            out=mT[:, :rows], in0=aggT[:, :rows],
                            in1=x0sT[:, r0:r0 + rows], op=mybir.AluOpType.add)
                        ps2 = ps.tile([P, P], f32, tag="dense", space="PSUM")
                        nc.tensor.matmul(out=ps2[:rows, :], lhsT=mT[:, :rows],
                                         rhs=what_s[:, l * D:(l + 1) * D],
                                         start=True, stop=True)
                        hb = hp.tile([P, D], f32 if is_last else fh, tag="hbo")
                        nc.scalar.activation(
                            out=hb[:rows, :], in_=ps2[:rows, :],
                            func=mybir.ActivationFunctionType.Relu)
                        dst_dram = out_t if is_last else shbuf[(l + 1) % 2]
                        nc.sync.dma_start(out=dst_dram[r0:r0 + rows, :],
                                          in_=hb[:rows, :])
                if not is_last:
                    if _AG_OFF:
                        nc.sync.dma_start(out=hbuf[(l + 1) % 2][0:ns, :],
                                          in_=shbuf[(l + 1) % 2][:])
                    else:
                        nc.gpsimd.collective_compute(
                            "AllGather", mybir.AluOpType.bypass,
                            replica_groups=rg,
                            ins=[shbuf[(l + 1) % 2][:]],
                            outs=[hbuf[(l + 1) % 2][:]])
    return nc


def _run(inputs, *, n, e, n_layers, ncores=NCORES):
    import concourse.bacc as bacc
    from concourse.bass_utils import run_bass_kernel_spmd

    x = np.asarray(inputs["x"], dtype=np.float32)
    edge_weight = np.asarray(inputs["edge_weight"], dtype=np.float32)
    w_lin = np.asarray(inputs["W_lin"], dtype=np.float32)
    b_lin = np.asarray(inputs["b_lin"], dtype=np.float32)
    w_convs = np.asarray(inputs["W_convs"], dtype=np.float32)
    edge_index = np.asarray(inputs["edge_index"])

    ns = n // ncores
    nb = -(-ns // P)

    betas = np.log(THETA / np.arange(1, n_layers + 1) + 1.0).astype(np.float32)
    eye = np.eye(D, dtype=np.float32)
    w_hat = np.stack([betas[l] * w_convs[l] + (1.0 - betas[l]) * eye
                      for l in range(n_layers)]).astype(np.float32)

    sel_arrs, idx_arrs, nch, gathers, block_chunks = \
        _preprocess(edge_index, edge_weight, n, ns, nb, ncores)

    key = (n, e, n_layers, nch, _SORT_SRC)
    if key not in _NEFF_CACHE:
        nc = bacc.Bacc("TRN2", target_bir_lowering=False, debug=False,
                       num_devices=ncores, num_swdge_queues=4,
                       dynamic_dma_scratch_size=32768)
        _build(nc, n=n, ns=ns, nb=nb, nch=nch, gathers=gathers,
               block_chunks=block_chunks, n_layers=n_layers)
        nc.compile()
        _NEFF_CACHE[key] = nc
    nc = _NEFF_CACHE[key]

    in_maps = []
    for c in range(ncores):
        in_maps.append({
            "x_shard": np.ascontiguousarray(x[c * ns:(c + 1) * ns]),
            "w_lin": w_lin, "b_lin": b_lin, "w_hat": w_hat,
            "sel": sel_arrs[c],
            "idx16": idx_arrs[c],
        })
    global _LAST_IN_MAPS
    _LAST_IN_MAPS = in_maps
    res = run_bass_kernel_spmd(nc, in_maps, list(range(ncores)))
    out = np.concatenate([res.results[c]["h_out"] for c in range(ncores)],
                         axis=0)
    return out


def kernel(x, edge_weight, W_lin, b_lin, W_convs, edge_index):
    return _run(
        dict(x=x, edge_weight=edge_weight, W_lin=W_lin, b_lin=b_lin,
             W_convs=W_convs, edge_index=edge_index),
        n=N, e=E, n_layers=L)


# revision 21
# speedup vs baseline: 1.0967x; 1.0865x over previous
"""GCNII (8-layer GCN2Conv stack) on 8 Trainium2 NeuronCores.

Strategy (1D graph parallel over destination nodes):
  - Core c owns destination rows [c*NS, (c+1)*NS) of every layer's output.
  - Edges are partitioned by dst, sorted by dst, grouped into per-128-dst-node
    "blocks" padded to 128-edge chunks (schedule shared across cores, SPMD);
    blocks are grouped into granules of GRAN blocks. Within each (block,
    src-half) group edges are ordered by source row so consecutive gather
    descriptors hit nearby HBM addresses.
  - One-hot selector tiles S[e, j] = (j == dstloc[e]) * 0.9*w[e] are constant
    across layers: precomputed on the HOST (bf16) and streamed from DRAM once
    per granule per layer (sync-engine DMA) instead of being rebuilt on the
    vector engine per chunk per layer.
  - Gathers are granule-batched: per (granule, src-half) one gpsimd.dma_gather
    pulls all the granule's source rows (128 rows x 256B, bf16) from the
    full-h DRAM buffer into SBUF; the lo/hi src-half split keeps gather
    indices within int16.
  - Per chunk: tensor-engine matmul aggT += V.T @ S accumulated in PSUM
    (feature-major [feat, dst]); per block: mT = aggT + 0.1*x0T (vector),
    node-major h' = relu(mT.T @ W_hat_l) via one matmul (W_hat_l =
    beta_l*W_l + (1-beta_l)*I folded on host; the GCNII (1-alpha) factor is
    folded into edge weights) + scalar-engine relu, DMA to the layer's
    shard buffer.
  - AllGather of the 6250-row shard between layers gives every core the full
    h for the next layer's gathers. Final layer writes the external output
    directly (host concatenates shards).
"""

import numpy as np

N = 50000
E = 800000
D = 128
L = 8
ALPHA = 0.1
THETA = 0.5
NCORES = 8
P = 128
LO_N = 32768
GRAN = 4

_NEFF_CACHE = {}
_LAST_IN_MAPS = None
_GATHER_MODE = "batch"
_SEL_DMA = True
_MM_ON = True
_AG_OFF = False
_H_BF16 = True
_VP_BUFS = 3
_SP_BUFS = 3
_PS_BUFS = 4
_SORT_SRC = True


def _preprocess(edge_index, edge_weight, n, ns, nb, ncores):
    import ml_dtypes

    src = np.asarray(edge_index[0], dtype=np.int64)
    dst = np.asarray(edge_index[1], dtype=np.int64)
    w = np.asarray(edge_weight, dtype=np.float32) * (1.0 - ALPHA)

    order = np.argsort(dst, kind="stable")
    src_s, dst_s, w_s = src[order], dst[order], w[order]

    core = dst_s // ns
    local = dst_s - core * ns
    block = local // P
    dstloc = (local - block * P).astype(np.int64)
    hi = (src_s >= LO_N).astype(np.int64)

    ngran = -(-nb // GRAN)

    cbh = (core * nb + block) * 2 + hi
    counts = np.bincount(cbh, minlength=ncores * nb * 2).reshape(ncores, nb, 2)
    cpg = -(-counts.max(axis=0) // P)
    cpg = np.maximum(cpg, 1)
    nch = int(cpg.sum())

    group_koff = np.zeros((nb, 2), dtype=np.int64)
    gathers = []
    k = 0
    for g in range(ngran):
        b0, b1 = g * GRAN, min((g + 1) * GRAN, nb)
        for h in range(2):
            k0 = k
            for b in range(b0, b1):
                group_koff[b, h] = k
                k += cpg[b, h]
            gathers.append((g, h, int(k0), int(k - k0)))
    assert k == nch
    block_chunks = {
        b: [(int(group_koff[b, 0]), int(cpg[b, 0])),
            (int(group_koff[b, 1]), int(cpg[b, 1]))]
        for b in range(nb)
    }

    sel_arrs, idx_arrs = [], []
    core_starts = np.searchsorted(core, np.arange(ncores + 1))
    for c in range(ncores):
        s0, s1 = core_starts[c], core_starts[c + 1]
        blk = block[s0:s1]
        bstart = np.searchsorted(blk, np.arange(nb + 1)) + s0
        da = np.zeros(nch * P, dtype=np.int64)
        wa = np.zeros(nch * P, dtype=np.float32)
        i16 = np.zeros(nch * P, dtype=np.int16)
        for b in range(nb):
            e0, e1 = bstart[b], bstart[b + 1]
            seg_src = src_s[e0:e1]
            seg_hi = seg_src >= LO_N
            for h in range(2):
                m = seg_hi if h else ~seg_hi
                gsrc = seg_src[m]
                if _SORT_SRC:
                    o = np.argsort(gsrc, kind="stable")
                else:
                    o = np.arange(len(gsrc))
                cnt = len(gsrc)
                pos = group_koff[b, h] * P
                da[pos:pos + cnt] = dstloc[e0:e1][m][o]
                wa[pos:pos + cnt] = w_s[e0:e1][m][o]
                i16[pos:pos + cnt] = (gsrc[o] - (LO_N if h else 0)).astype(
                    np.int16)
        s3 = np.zeros((nch, P, P), dtype=np.float32)
        kk = np.repeat(np.arange(nch), P)
        pp = np.tile(np.arange(P), nch)
        s3[kk, pp, da] = wa
        sel = np.ascontiguousarray(
            s3.transpose(1, 0, 2).reshape(P, nch * P)).astype(
                ml_dtypes.float8_e4m3)
        sel_arrs.append(sel)
        ia = np.zeros((P, nch * 8), dtype=np.int16)
        for (_, _, k0, m) in gathers:
            sub = i16[k0 * P:(k0 + m) * P]
            ia[:16, k0 * 8:(k0 + m) * 8] = sub.reshape(m * 8, 16).T
        ia = np.tile(ia[:16], (8, 1))
        idx_arrs.append(np.ascontiguousarray(ia))

    return sel_arrs, idx_arrs, nch, gathers, block_chunks


def _build(nc, *, n, ns, nb, nch, gathers, block_chunks, n_layers):
    import concourse.mybir as mybir
    import concourse.tile as tile
    from concourse.masks import make_identity

    f32 = mybir.dt.float32
    fh = mybir.dt.bfloat16 if _H_BF16 else f32

    x_t = nc.dram_tensor("x_shard", [ns, D], f32, kind="ExternalInput")
    wlin_t = nc.dram_tensor("w_lin", [D, D], f32, kind="ExternalInput")
    blin_t = nc.dram_tensor("b_lin", [D], f32, kind="ExternalInput")
    what_t = nc.dram_tensor("w_hat", [n_layers, D, D], f32, kind="ExternalInput")
    f8 = mybir.dt.float8e4
    sel_t = nc.dram_tensor("sel", [P, nch * P], f8, kind="ExternalInput")
    idx16_t = nc.dram_tensor("idx16", [P, nch * 8], mybir.dt.int16,
                             kind="ExternalInput")
    out_t = nc.dram_tensor("h_out", [ns, D], f32, kind="ExternalOutput")

    hbuf = [nc.dram_tensor(f"h_full{i}", [n, D], fh, addr_space="Shared")
            for i in range(2)]
    shbuf = [nc.dram_tensor(f"h_shard{i}", [ns, D], fh) for i in range(2)]
    rg = [list(range(NCORES))]

    n_full = ns // P
    last = ns - n_full * P
    blk_rows = [P] * n_full + ([last] if last else [])
    assert len(blk_rows) == nb

    gran_of = {}
    for (g, h, k0, m) in gathers:
        gran_of.setdefault(g, []).append((h, k0, m))
    ngran = len(gran_of)
    m_max = [max(m for (h2, _, m) in sum(gran_of.values(), []) if h2 == h)
             for h in range(2)]
    gsel = []
    for g in range(ngran):
        ks = [(k0, m) for (_, k0, m) in gran_of[g]]
        gsel.append((min(k for k, _ in ks), sum(m for _, m in ks)))
    msel_max = max(m for _, m in gsel)

    with tile.TileContext(nc) as tc:
        with (
            tc.tile_pool(name="res", bufs=1) as res,
            tc.tile_pool(name="xp", bufs=3) as xp,
            tc.tile_pool(name="vp", bufs=_VP_BUFS) as vp,
            tc.tile_pool(name="sp", bufs=_SP_BUFS) as sp,
            tc.tile_pool(name="mp", bufs=4) as mp,
            tc.tile_pool(name="hp", bufs=6) as hp,
            tc.tile_pool(name="ps", bufs=2, space="PSUM") as ps,
            tc.tile_pool(name="psa", bufs=_PS_BUFS, space="PSUM") as psa,
        ):
            ident = res.tile([P, P], f32, tag="ident")
            make_identity(nc, ident[:])

            wlin_s = res.tile([P, D], f32, tag="wlin")
            nc.sync.dma_start(out=wlin_s[:], in_=wlin_t[:])
            blin_s = res.tile([P, 1], f32, tag="blin")
            nc.sync.dma_start(out=blin_s[:], in_=blin_t[:, None])
            what_s = res.tile([P, n_layers * D], f32, tag="what")
            for l in range(n_layers):
                nc.sync.dma_start(out=what_s[:, l * D:(l + 1) * D],
                                  in_=what_t[l, :, :])
            idx16 = res.tile([P, nch * 8], mybir.dt.int16, tag="idx16")
            nc.sync.dma_start(out=idx16[:], in_=idx16_t[:])

            x0sT = res.tile([P, ns], f32, tag="x0sT")

            sel_const = None
            if not _SEL_DMA:
                sel_const = res.tile([P, P], f8, tag="selc")
                nc.vector.memset(sel_const[:], 0.0)
            v_const = None
            if _GATHER_MODE == "off":
                v_const = res.tile([P, 1, D], fh, tag="vc")
                nc.vector.memset(v_const[:], 0.0)

            for b in range(nb):
                rows = blk_rows[b]
                r0 = b * P
                xb = xp.tile([P, D], f32, tag="xb")
                nc.sync.dma_start(out=xb[:rows, :], in_=x_t[r0:r0 + rows, :])
                xbT_ps = ps.tile([P, P], f32, tag="tr", space="PSUM")
                nc.tensor.transpose(out=xbT_ps[:, :rows], in_=xb[:rows, :],
                                    identity=ident[:rows, :rows])
                xbT = xp.tile([P, P], f32, tag="xbT")
                nc.vector.tensor_copy(out=xbT[:, :rows], in_=xbT_ps[:, :rows])
                ps2 = ps.tile([P, P], f32, tag="dense", space="PSUM")
                nc.tensor.matmul(out=ps2[:, :rows], lhsT=wlin_s[:],
                                 rhs=xbT[:, :rows], start=True, stop=True)
                x0Tb = xp.tile([P, P], f32, tag="x0Tb")
                nc.scalar.activation(out=x0Tb[:, :rows], in_=ps2[:, :rows],
                                     func=mybir.ActivationFunctionType.Relu,
                                     bias=blin_s[:, :1], scale=1.0)
                nc.vector.tensor_scalar(
                    out=x0sT[:, r0:r0 + rows], in0=x0Tb[:, :rows],
                    scalar1=ALPHA, scalar2=None, op0=mybir.AluOpType.mult)
                x0_ps = ps.tile([P, P], f32, tag="tr", space="PSUM")
                nc.tensor.transpose(out=x0_ps[:rows, :], in_=x0Tb[:, :rows],
                                    identity=ident[:])
                x0b = hp.tile([P, D], fh, tag="hb")
                nc.vector.tensor_copy(out=x0b[:rows, :], in_=x0_ps[:rows, :])
                nc.sync.dma_start(out=shbuf[0][r0:r0 + rows, :],
                                  in_=x0b[:rows, :])

            if _AG_OFF:
                nc.sync.dma_start(out=hbuf[0][0:ns, :], in_=shbuf[0][:])
            else:
                nc.gpsimd.collective_compute(
                    "AllGather", mybir.AluOpType.bypass, replica_groups=rg,
                    ins=[shbuf[0][:]], outs=[hbuf[0][:]])

            gq = 0
            for l in range(n_layers):
                h_cur = hbuf[l % 2]
                is_last = l == n_layers - 1
                for g in range(ngran):
                    b0, b1 = g * GRAN, min((g + 1) * GRAN, nb)
                    k0s, msel = gsel[g]
                    if _SEL_DMA:
                        selg = sp.tile([P, msel_max * P], f8, tag="sel")
                        nc.sync.dma_start(
                            out=selg[:, :msel * P],
                            in_=sel_t[:, k0s * P:(k0s + msel) * P])
                    vts = {}
                    for (h, k0, m) in gran_of[g]:
                        vt = vp.tile([P, m_max[h], D], fh, tag=f"v{h}")
                        vts[h] = (vt, k0)
                        if _GATHER_MODE == "batch":
                            in_ap = (h_cur[LO_N:n, :] if h
                                     else h_cur[0:LO_N, :])
                            nc.gpsimd.dma_gather(
                                vt[:, :m, :], in_ap,
                                idx16[:, k0 * 8:(k0 + m) * 8],
                                num_idxs=m * P, num_idxs_reg=m * P,
                                elem_size=D, single_packet=False,
                                queue_num=gq % 4)
                            gq += 1
                    for b in range(b0, b1):
                        rows = blk_rows[b]
                        r0 = b * P
                        nk = sum(m for _, m in block_chunks[b])
                        aggT = psa.tile([P, P], f32, tag="agg", space="PSUM")
                        ki = 0
                        for h in range(2):
                            kb, mb = block_chunks[b][h]
                            vt, kg = vts[h]
                            for j in range(mb):
                                kchunk = kb + j
                                if _SEL_DMA:
                                    s_ap = selg[:, (kchunk - k0s) * P:
                                                (kchunk - k0s + 1) * P]
                                else:
                                    s_ap = sel_const[:]
                                v_ap = (v_const[:, 0, :] if v_const is not None
                                        else vt[:, kchunk - kg, :])
                                if _MM_ON:
                                    nc.tensor.matmul(
                                        out=aggT[:], lhsT=v_ap, rhs=s_ap,
                                        start=(ki == 0), stop=(ki == nk - 1))
                                ki += 1
                        if not _MM_ON:
                            nc.vector.memset(aggT[:], 0.0)
                        mT = mp.tile([P, P], f32, tag="mT")
                        nc.vector.tensor_tensor(
                            out=mT[:, :rows], in0=aggT[:, :rows],
                            in1=x0sT[:, r0:r0 + rows], op=mybir.AluOpType.add)
                        ps2 = ps.tile([P, P], f32, tag="dense", space="PSUM")
                        nc.tensor.matmul(out=ps2[:rows, :], lhsT=mT[:, :rows],
                                         rhs=what_s[:, l * D:(l + 1) * D],
                                         start=True, stop=True)
                        hb = hp.tile([P, D], f32 if is_last else fh, tag="hbo")
                        nc.scalar.activation(
                            out=hb[:rows, :], in_=ps2[:rows, :],
                            func=mybir.ActivationFunctionType.Relu)
                        dst_dram = out_t if is_last else shbuf[(l + 1) % 2]
                        nc.sync.dma_start(out=dst_dram[r0:r0 + rows, :],
                                          in_=hb[:rows, :])
                if not is_last:
                    if _AG_OFF:
                        nc.sync.dma_start(out=hbuf[(l + 1) % 2][0:ns, :],
                                          in_=shbuf[(l + 1) % 2][:])
                    else:
                        nc.gpsimd.collective_compute(
                            "AllGather", mybir.AluOpType.bypass,
                            replica_groups=rg,
                            ins=[shbuf[(l + 1) % 2][:]],
                            outs=[hbuf[(l + 1) % 2][:]])
    return nc


def _run(inputs, *, n, e, n_layers, ncores=NCORES):
    import concourse.bacc as bacc
    from concourse.bass_utils import run_bass_kernel_spmd

    x = np.asarray(inputs["x"], dtype=np.float32)
    edge_weight = np.asarray(inputs["edge_weight"], dtype=np.float32)
    w_lin = np.asarray(inputs["W_lin"], dtype=np.float32)
    b_lin = np.asarray(inputs["b_lin"], dtype=np.float32)
    w_convs = np.asarray(inputs["W_convs"], dtype=np.float32)
    edge_index = np.asarray(inputs["edge_index"])

    ns = n // ncores
    nb = -(-ns // P)

    betas = np.log(THETA / np.arange(1, n_layers + 1) + 1.0).astype(np.float32)
    eye = np.eye(D, dtype=np.float32)
    w_hat = np.stack([betas[l] * w_convs[l] + (1.0 - betas[l]) * eye
                      for l in range(n_layers)]).astype(np.float32)

    sel_arrs, idx_arrs, nch, gathers, block_chunks = \
        _preprocess(edge_index, edge_weight, n, ns, nb, ncores)

    key = (n, e, n_layers, nch, _SORT_SRC)
    if key not in _NEFF_CACHE:
        nc = bacc.Bacc("TRN2", target_bir_lowering=False, debug=False,
                       num_devices=ncores, num_swdge_queues=4,
                       dynamic_dma_scratch_size=32768)
        _build(nc, n=n, ns=ns, nb=nb, nch=nch, gathers=gathers,
               block_chunks=block_chunks, n_layers=n_layers)
        nc.compile()
        _NEFF_CACHE[key] = nc
    nc = _NEFF_CACHE[key]

    in_maps = []
    for c in range(ncores):
        in_maps.append({
            "x_shard": np.ascontiguousarray(x[c * ns:(c + 1) * ns]),
            "w_lin": w_lin, "b_lin": b_lin, "w_hat": w_hat,
            "sel": sel_arrs[c],
            "idx16": idx_arrs[c],
        })
    global _LAST_IN_MAPS
    _LAST_IN_MAPS = in_maps
    res = run_bass_kernel_spmd(nc, in_maps, list(range(ncores)))
    out = np.concatenate([res.results[c]["h_out"] for c in range(ncores)],
                         axis=0)
    return out


def kernel(x, edge_weight, W_lin, b_lin, W_convs, edge_index):
    return _run(
        dict(x=x, edge_weight=edge_weight, W_lin=W_lin, b_lin=b_lin,
             W_convs=W_convs, edge_index=edge_index),
        n=N, e=E, n_layers=L)
